# revision 19
# baseline (speedup 1.0000x reference)
"""DIEN GRU-with-attention kernel for Trainium2 (8 NeuronCores, Bass/Tile).

Math note: the reference computes softmax over a singleton axis, which is
exactly 1.0, so the attention branch (Wa, item) never affects the output.
The computation reduces to a plain GRU:
    u  = sigmoid(x_t @ Wu + h @ Uu + bu)
    r  = sigmoid(x_t @ Wr + h @ Ur + br)
    hh = tanh(x_t @ Wh + r * (h @ Uh) + bh)
    h' = (1 - u) * h + u * hh

Device layout is feature-major: tiles are [D=128 partitions, batch free].
The batch dim (2048) is sharded 8 ways (256 per core); the host does the
[batch, feat] <-> [feat, batch] layout transposes during shard/unshard.
"""

import sys

if "/opt/trn_rl_repo" not in sys.path:
    sys.path.insert(0, "/opt/trn_rl_repo")

from concurrent.futures import ThreadPoolExecutor
from contextlib import ExitStack

import os

import numpy as np

B, S, D = 2048, 200, 128
NCORES = 8
BS = B // NCORES  # batch per core

# float32r streams the PE at 1 cycle/row (vs 4 for fp32's two-pass lowering)
# but measured 5.6e-2 rel err end-to-end — too lossy. Default off.
USE_F32R = os.environ.get("DIEN_F32R", "0") == "1"

_BUILT = None  # cached compiled module


def _body(ctx, tc, aps, n_steps):
    import concourse.bass as bass  # noqa: F401
    from concourse import mybir

    nc = tc.nc
    f32 = mybir.dt.float32
    Sigmoid = mybir.ActivationFunctionType.Sigmoid
    Tanh = mybir.ActivationFunctionType.Tanh

    fmm = mybir.dt.float32r if USE_F32R else f32

    singles = ctx.enter_context(tc.tile_pool(name="singles", bufs=1))
    xpool = ctx.enter_context(tc.tile_pool(name="xp", bufs=4))
    hpool = ctx.enter_context(tc.tile_pool(name="hp", bufs=3))
    tmp = ctx.enter_context(tc.tile_pool(name="tmp", bufs=3))
    p_ur_pool = ctx.enter_context(tc.tile_pool(name="p_ur", bufs=2, space="PSUM"))
    p_g_pool = ctx.enter_context(tc.tile_pool(name="p_g", bufs=2, space="PSUM"))

    W = {}
    for name in ("Wu", "Wr", "Wh", "Uu", "Ur", "Uh"):
        t = singles.tile([D, D], fmm, tag=name)
        nc.sync.dma_start(t[:], aps[name])
        W[name] = t
    Bv = {}
    for name in ("bu", "br", "bh", "nbu"):
        t = singles.tile([D, 1], f32, tag=name)
        nc.sync.dma_start(t[:], aps[name])
        Bv[name] = t

    h = hpool.tile([D, BS], fmm, tag="h")
    nc.sync.dma_start(h[:], aps["h0T"])

    xT = aps["xT"]
    outT = aps["outT"]

    assert n_steps % 2 == 0
    for tp in range(n_steps // 2):
        t0 = 2 * tp
        # two steps' x tiles side by side: [D, 2, BS] -> N=512 matmuls
        xt = xpool.tile([D, 2, BS], fmm, tag="x")
        nc.sync.dma_start(xt[:], xT[t0 : t0 + 2].rearrange("s d b -> d s b"))

        p_u = p_ur_pool.tile([D, 2, BS], f32, tag="pu")
        p_r = p_ur_pool.tile([D, 2, BS], f32, tag="pr")
        p_zh = p_g_pool.tile([D, 2, BS], f32, tag="zh")

        # x projections for both steps in one N=512 matmul per weight.
        # They start each PSUM accumulation group; the per-step h-matmuls
        # accumulate into their half-bank slice afterwards.
        nc.tensor.matmul(p_r[:], W["Wr"][:], xt[:], start=True, stop=False, skip_group_check=True)
        nc.tensor.matmul(p_zh[:], W["Wh"][:], xt[:], start=True, stop=True)
        nc.tensor.matmul(p_u[:], W["Wu"][:], xt[:], start=True, stop=False, skip_group_check=True)

        for s in (0, 1):
            t_step = t0 + s
            last = s == 1
            p_hh = p_g_pool.tile([D, BS], f32, tag="hh")

            # h projections: mm_hr heads the serial chain (sigma_r -> m ->
            # z -> tanh), so issue it first once h is ready.
            nc.tensor.matmul(p_r[:, s], W["Ur"][:], h[:], start=False, stop=last, skip_group_check=True)
            nc.tensor.matmul(p_hh[:], W["Uh"][:], h[:], start=True, stop=True)
            nc.tensor.matmul(p_u[:, s], W["Uu"][:], h[:], start=False, stop=last, skip_group_check=True)

            r = tmp.tile([D, BS], f32, tag="r")
            nc.scalar.activation(r[:], p_r[:, s], Sigmoid, bias=Bv["br"][:])
            # um = 1-u = sigmoid(-(zu+bu)); same PSUM bank, negated scale/bias
            um = tmp.tile([D, BS], f32, tag="um")
            nc.scalar.activation(
                um[:], p_u[:, s], Sigmoid, bias=Bv["nbu"][:], scale=-1.0
            )
            u = tmp.tile([D, BS], f32, tag="u")
            nc.scalar.activation(u[:], p_u[:, s], Sigmoid, bias=Bv["bu"][:])

            m = tmp.tile([D, BS], f32, tag="m")
            nc.vector.tensor_mul(m[:], r[:], p_hh[:])
            z = tmp.tile([D, BS], f32, tag="z")
            nc.vector.tensor_add(z[:], m[:], p_zh[:, s])

            hh = tmp.tile([D, BS], f32, tag="hh")
            nc.scalar.activation(hh[:], z[:], Tanh, bias=Bv["bh"][:])

            # blend: h' = (1-u)*h + u*hh; q1 runs off the tanh chain
            q1 = tmp.tile([D, BS], f32, tag="q1")
            nc.vector.tensor_mul(q1[:], um[:], h[:])
            q2 = tmp.tile([D, BS], f32, tag="q2")
            nc.vector.tensor_mul(q2[:], u[:], hh[:])
            h_new = hpool.tile([D, BS], fmm, tag="h")
            nc.vector.tensor_add(h_new[:], q1[:], q2[:])

            nc.sync.dma_start(outT[t_step], h_new[:])
            h = h_new


def build_module(n_steps=S):
    import concourse.bacc as bacc
    import concourse.tile as tile
    from concourse import mybir

    f32 = mybir.dt.float32
    fmm = mybir.dt.float32r if USE_F32R else f32
    nc = bacc.Bacc(
        "TRN2",
        target_bir_lowering=False,
        debug=False,
        enable_asserts=False,
        num_devices=NCORES,
    )

    aps = {}
    aps["xT"] = nc.dram_tensor("xT", [n_steps, D, BS], fmm, kind="ExternalInput").ap()
    aps["h0T"] = nc.dram_tensor("h0T", [D, BS], fmm, kind="ExternalInput").ap()
    for name in ("Wu", "Wr", "Wh", "Uu", "Ur", "Uh"):
        aps[name] = nc.dram_tensor(name, [D, D], fmm, kind="ExternalInput").ap()
    for name in ("bu", "br", "bh", "nbu"):
        aps[name] = nc.dram_tensor(name, [D, 1], f32, kind="ExternalInput").ap()
    aps["outT"] = nc.dram_tensor(
        "outT", [n_steps, D, BS], fmm, kind="ExternalOutput"
    ).ap()

    with tile.TileContext(nc) as tc, ExitStack() as ctx:
        _body(ctx, tc, aps, n_steps)
    nc.compile()
    return nc


def _get_built():
    global _BUILT
    if _BUILT is None:
        _BUILT = build_module(S)
    return _BUILT


def _shard_core(c, x, h0):
    xc = x[c * BS : (c + 1) * BS]  # [BS, S, D]
    xT = np.empty((S, D, BS), dtype=np.float32)
    for t in range(S):
        xT[t] = xc[:, t, :].T
    h0T = np.ascontiguousarray(h0[c * BS : (c + 1) * BS].T)
    return xT, h0T


def _prep_in_maps(inputs):
    x = np.ascontiguousarray(np.asarray(inputs["x"], dtype=np.float32))
    h0 = np.ascontiguousarray(np.asarray(inputs["h0"], dtype=np.float32))
    shared = {}
    for name in ("Wu", "Wr", "Wh", "Uu", "Ur", "Uh"):
        shared[name] = np.ascontiguousarray(np.asarray(inputs[name], dtype=np.float32))
    for name in ("bu", "br", "bh"):
        shared[name] = np.ascontiguousarray(
            np.asarray(inputs[name], dtype=np.float32).reshape(1, D).T
        )
    shared["nbu"] = np.ascontiguousarray(-shared["bu"])
    with ThreadPoolExecutor(NCORES) as ex:
        parts = list(ex.map(lambda c: _shard_core(c, x, h0), range(NCORES)))

    in_maps = []
    for c in range(NCORES):
        xT, h0T = parts[c]
        m = {"xT": xT, "h0T": h0T}
        m.update(shared)
        in_maps.append(m)
    return in_maps


def _unshard_core(c, outT, outs):
    # outT: [S, D, BS] -> outs[c*BS:(c+1)*BS] = [BS, S, D]
    dst = outs[c * BS : (c + 1) * BS]
    for t in range(S):
        dst[:, t, :] = outT[t].T


def _assemble(results):
    outs = np.empty((B, S, D), dtype=np.float32)
    with ThreadPoolExecutor(NCORES) as ex:
        list(
            ex.map(
                lambda c: _unshard_core(c, results[c]["outT"], outs), range(NCORES)
            )
        )
    h_last = np.ascontiguousarray(outs[:, -1, :])
    return outs, h_last


def _ensure_ntff_hook():
    """Install the axon NTFF profile hook if the image's antenv lacks it."""
    try:
        from antenv.axon_hooks import get_axon_ntff_profile_hook  # noqa: F401

        return True
    except ImportError:
        pass
    try:
        import types

        import antenv
        from trn_agent_boot.trn_boot import _ntff_profile_via_ctypes

        hook = _ntff_profile_via_ctypes("/opt/axon/libaxon_pjrt.so")
        mod = types.ModuleType("antenv.axon_hooks")
        mod.get_axon_ntff_profile_hook = lambda: hook
        mod.set_axon_ntff_profile_hook = lambda h: None
        sys.modules["antenv.axon_hooks"] = mod
        antenv.axon_hooks = mod
        return hook is not None
    except Exception as e:  # pragma: no cover
        print(f"NTFF hook install failed: {e}", file=sys.stderr)
        return False


def run(inputs, trace=False):
    """Run on hardware; returns ((outs, h_last), exec_time_ns_or_None)."""
    import concourse.bass_utils as bass_utils

    if trace:
        _ensure_ntff_hook()
        bass_utils.upload_artifacts = lambda tmpdir: f"local:{tmpdir}"

    nc = _get_built()
    in_maps = _prep_in_maps(inputs)
    res = bass_utils.run_bass_kernel_spmd(
        nc, in_maps, core_ids=list(range(NCORES)), trace=trace
    )
    return _assemble(res.results), res.exec_time_ns


def kernel(**inputs):
    (outs, h_last), _ = run(inputs, trace=False)
    return outs, h_last


# revision 22
# speedup vs baseline: 1.1942x; 1.1942x over previous
"""DIEN GRU-with-attention kernel for Trainium2 (8 NeuronCores, Bass/Tile).

Math note: the reference computes softmax over a singleton axis, which is
exactly 1.0, so the attention branch (Wa, item) never affects the output.
The computation reduces to a plain GRU:
    u  = sigmoid(x_t @ Wu + h @ Uu + bu)
    r  = sigmoid(x_t @ Wr + h @ Ur + br)
    hh = tanh(x_t @ Wh + r * (h @ Uh) + bh)
    h' = (1 - u) * h + u * hh

Device layout is feature-major: tiles are [D=128 partitions, batch free].
The batch dim (2048) is sharded 8 ways (256 per core); the host does the
[batch, feat] <-> [feat, batch] layout transposes during shard/unshard.
"""

import sys

if "/opt/trn_rl_repo" not in sys.path:
    sys.path.insert(0, "/opt/trn_rl_repo")

from concurrent.futures import ThreadPoolExecutor
from contextlib import ExitStack

import os

import numpy as np

B, S, D = 2048, 200, 128
NCORES = 8
BS = B // NCORES  # batch per core

# float32r streams the PE at 1 cycle/row (vs 4 for fp32's two-pass lowering)
# but measured 5.6e-2 rel err end-to-end — too lossy. Default off.
USE_F32R = os.environ.get("DIEN_F32R", "0") == "1"

_BUILT = None  # cached compiled module


def _body(ctx, tc, aps, n_steps):
    import concourse.bass as bass  # noqa: F401
    from concourse import mybir

    nc = tc.nc
    f32 = mybir.dt.float32
    Sigmoid = mybir.ActivationFunctionType.Sigmoid
    Tanh = mybir.ActivationFunctionType.Tanh

    fmm = mybir.dt.float32r if USE_F32R else f32

    singles = ctx.enter_context(tc.tile_pool(name="singles", bufs=1))
    xpool = ctx.enter_context(tc.tile_pool(name="xp", bufs=4))
    hpool = ctx.enter_context(tc.tile_pool(name="hp", bufs=3))
    tmp = ctx.enter_context(tc.tile_pool(name="tmp", bufs=3))
    p_ur_pool = ctx.enter_context(tc.tile_pool(name="p_ur", bufs=2, space="PSUM"))
    p_g_pool = ctx.enter_context(tc.tile_pool(name="p_g", bufs=2, space="PSUM"))

    W = {}
    for name in ("Wu", "Wr", "Wh", "Uu", "Ur", "Uh"):
        t = singles.tile([D, D], fmm, tag=name)
        nc.sync.dma_start(t[:], aps[name])
        W[name] = t
    Bv = {}
    for name in ("bu", "br", "bh", "nbu"):
        t = singles.tile([D, 1], f32, tag=name)
        nc.sync.dma_start(t[:], aps[name])
        Bv[name] = t

    h = hpool.tile([D, BS], fmm, tag="h")
    nc.sync.dma_start(h[:], aps["h0T"])

    xT = aps["xT"]
    outT = aps["outT"]

    for t_step in range(n_steps):
        xt = xpool.tile([D, BS], fmm, tag="x")
        nc.sync.dma_start(xt[:], xT[t_step])

        p_u = p_ur_pool.tile([D, BS], f32, tag="pu")
        p_r = p_ur_pool.tile([D, BS], f32, tag="pr")
        p_zh = p_g_pool.tile([D, BS], f32, tag="zh")
        p_hh = p_g_pool.tile([D, BS], f32, tag="hh")

        # x projections (independent of h; start each PSUM group, so they
        # must execute before the matching h-matmul accumulate)
        nc.tensor.matmul(p_r[:], W["Wr"][:], xt[:], start=True, stop=False)
        nc.tensor.matmul(p_zh[:], W["Wh"][:], xt[:], start=True, stop=True)
        nc.tensor.matmul(p_u[:], W["Wu"][:], xt[:], start=True, stop=False)
        # h projections: mm_hr heads the serial chain (sigma_r -> m -> z ->
        # tanh), so issue it first once h is ready; mm_hh feeds m next.
        nc.tensor.matmul(p_r[:], W["Ur"][:], h[:], start=False, stop=True)
        nc.tensor.matmul(p_hh[:], W["Uh"][:], h[:], start=True, stop=True)
        nc.tensor.matmul(p_u[:], W["Uu"][:], h[:], start=False, stop=True)

        r = tmp.tile([D, BS], f32, tag="r")
        nc.scalar.activation(r[:], p_r[:], Sigmoid, bias=Bv["br"][:])
        u = tmp.tile([D, BS], f32, tag="u")
        nc.scalar.activation(u[:], p_u[:], Sigmoid, bias=Bv["bu"][:])

        m = tmp.tile([D, BS], f32, tag="m")
        nc.vector.tensor_mul(m[:], r[:], p_hh[:])
        z = tmp.tile([D, BS], f32, tag="z")
        nc.vector.tensor_add(z[:], m[:], p_zh[:])

        hh = tmp.tile([D, BS], f32, tag="hh")
        nc.scalar.activation(hh[:], z[:], Tanh, bias=Bv["bh"][:])

        # blend: h' = (1-u)*h + u*hh; um/q1 run off the tanh chain.
        # um = 1 - u on the DVE (tensor_scalar, 2x mode) keeps ACT free so
        # tanh isn't delayed behind a third sigmoid.
        um = tmp.tile([D, BS], f32, tag="um")
        nc.vector.tensor_scalar(
            um[:], u[:], -1.0, 1.0, mybir.AluOpType.mult, mybir.AluOpType.add
        )
        q1 = tmp.tile([D, BS], f32, tag="q1")
        nc.vector.tensor_mul(q1[:], um[:], h[:])
        q2 = tmp.tile([D, BS], f32, tag="q2")
        nc.vector.tensor_mul(q2[:], u[:], hh[:])
        h_new = hpool.tile([D, BS], fmm, tag="h")
        nc.vector.tensor_add(h_new[:], q1[:], q2[:])

        nc.sync.dma_start(outT[t_step], h_new[:])
        h = h_new


def build_module(n_steps=S):
    import concourse.bacc as bacc
    import concourse.tile as tile
    from concourse import mybir

    f32 = mybir.dt.float32
    fmm = mybir.dt.float32r if USE_F32R else f32
    nc = bacc.Bacc(
        "TRN2",
        target_bir_lowering=False,
        debug=False,
        enable_asserts=False,
        num_devices=NCORES,
    )

    aps = {}
    aps["xT"] = nc.dram_tensor("xT", [n_steps, D, BS], fmm, kind="ExternalInput").ap()
    aps["h0T"] = nc.dram_tensor("h0T", [D, BS], fmm, kind="ExternalInput").ap()
    for name in ("Wu", "Wr", "Wh", "Uu", "Ur", "Uh"):
        aps[name] = nc.dram_tensor(name, [D, D], fmm, kind="ExternalInput").ap()
    for name in ("bu", "br", "bh", "nbu"):
        aps[name] = nc.dram_tensor(name, [D, 1], f32, kind="ExternalInput").ap()
    aps["outT"] = nc.dram_tensor(
        "outT", [n_steps, D, BS], fmm, kind="ExternalOutput"
    ).ap()

    with tile.TileContext(nc) as tc, ExitStack() as ctx:
        _body(ctx, tc, aps, n_steps)
    nc.compile()
    return nc


def _get_built():
    global _BUILT
    if _BUILT is None:
        _BUILT = build_module(S)
    return _BUILT


def _shard_core(c, x, h0):
    xc = x[c * BS : (c + 1) * BS]  # [BS, S, D]
    xT = np.empty((S, D, BS), dtype=np.float32)
    for t in range(S):
        xT[t] = xc[:, t, :].T
    h0T = np.ascontiguousarray(h0[c * BS : (c + 1) * BS].T)
    return xT, h0T


def _prep_in_maps(inputs):
    x = np.ascontiguousarray(np.asarray(inputs["x"], dtype=np.float32))
    h0 = np.ascontiguousarray(np.asarray(inputs["h0"], dtype=np.float32))
    shared = {}
    for name in ("Wu", "Wr", "Wh", "Uu", "Ur", "Uh"):
        shared[name] = np.ascontiguousarray(np.asarray(inputs[name], dtype=np.float32))
    for name in ("bu", "br", "bh"):
        shared[name] = np.ascontiguousarray(
            np.asarray(inputs[name], dtype=np.float32).reshape(1, D).T
        )
    shared["nbu"] = np.ascontiguousarray(-shared["bu"])
    with ThreadPoolExecutor(NCORES) as ex:
        parts = list(ex.map(lambda c: _shard_core(c, x, h0), range(NCORES)))

    in_maps = []
    for c in range(NCORES):
        xT, h0T = parts[c]
        m = {"xT": xT, "h0T": h0T}
        m.update(shared)
        in_maps.append(m)
    return in_maps


def _unshard_core(c, outT, outs):
    # outT: [S, D, BS] -> outs[c*BS:(c+1)*BS] = [BS, S, D]
    dst = outs[c * BS : (c + 1) * BS]
    for t in range(S):
        dst[:, t, :] = outT[t].T


def _assemble(results):
    outs = np.empty((B, S, D), dtype=np.float32)
    with ThreadPoolExecutor(NCORES) as ex:
        list(
            ex.map(
                lambda c: _unshard_core(c, results[c]["outT"], outs), range(NCORES)
            )
        )
    h_last = np.ascontiguousarray(outs[:, -1, :])
    return outs, h_last


def _ensure_ntff_hook():
    """Install the axon NTFF profile hook if the image's antenv lacks it."""
    try:
        from antenv.axon_hooks import get_axon_ntff_profile_hook  # noqa: F401

        return True
    except ImportError:
        pass
    try:
        import types

        import antenv
        from trn_agent_boot.trn_boot import _ntff_profile_via_ctypes

        hook = _ntff_profile_via_ctypes("/opt/axon/libaxon_pjrt.so")
        mod = types.ModuleType("antenv.axon_hooks")
        mod.get_axon_ntff_profile_hook = lambda: hook
        mod.set_axon_ntff_profile_hook = lambda h: None
        sys.modules["antenv.axon_hooks"] = mod
        antenv.axon_hooks = mod
        return hook is not None
    except Exception as e:  # pragma: no cover
        print(f"NTFF hook install failed: {e}", file=sys.stderr)
        return False


def run(inputs, trace=False):
    """Run on hardware; returns ((outs, h_last), exec_time_ns_or_None)."""
    import concourse.bass_utils as bass_utils

    if trace:
        _ensure_ntff_hook()
        bass_utils.upload_artifacts = lambda tmpdir: f"local:{tmpdir}"

    nc = _get_built()
    in_maps = _prep_in_maps(inputs)
    res = bass_utils.run_bass_kernel_spmd(
        nc, in_maps, core_ids=list(range(NCORES)), trace=trace
    )
    return _assemble(res.results), res.exec_time_ns


def kernel(**inputs):
    (outs, h_last), _ = run(inputs, trace=False)
    return outs, h_last


# revision 24
# speedup vs baseline: 1.2847x; 1.0758x over previous
"""DIEN GRU-with-attention kernel for Trainium2 (8 NeuronCores, Bass/Tile).

Math note: the reference computes softmax over a singleton axis, which is
exactly 1.0, so the attention branch (Wa, item) never affects the output.
The computation reduces to a plain GRU:
    u  = sigmoid(x_t @ Wu + h @ Uu + bu)
    r  = sigmoid(x_t @ Wr + h @ Ur + br)
    hh = tanh(x_t @ Wh + r * (h @ Uh) + bh)
    h' = (1 - u) * h + u * hh

Device layout is feature-major: tiles are [D=128 partitions, batch free].
The batch dim (2048) is sharded 8 ways (256 per core); the host does the
[batch, feat] <-> [feat, batch] layout transposes during shard/unshard.
"""

import sys

if "/opt/trn_rl_repo" not in sys.path:
    sys.path.insert(0, "/opt/trn_rl_repo")

from concurrent.futures import ThreadPoolExecutor
from contextlib import ExitStack

import os

import numpy as np

B, S, D = 2048, 200, 128
NCORES = 8
BS = B // NCORES  # batch per core

# float32r streams the PE at 1 cycle/row (vs 4 for fp32's two-pass lowering)
# but measured 5.6e-2 rel err end-to-end — too lossy. Default off.
USE_F32R = os.environ.get("DIEN_F32R", "0") == "1"

_BUILT = None  # cached compiled module


def _body(ctx, tc, aps, n_steps):
    import concourse.bass as bass  # noqa: F401
    from concourse import mybir

    nc = tc.nc
    f32 = mybir.dt.float32
    Sigmoid = mybir.ActivationFunctionType.Sigmoid
    Tanh = mybir.ActivationFunctionType.Tanh

    fmm = mybir.dt.float32r if USE_F32R else f32

    singles = ctx.enter_context(tc.tile_pool(name="singles", bufs=1))
    xpool = ctx.enter_context(tc.tile_pool(name="xp", bufs=4))
    hpool = ctx.enter_context(tc.tile_pool(name="hp", bufs=3))
    tmp = ctx.enter_context(tc.tile_pool(name="tmp", bufs=3))
    p_ur_pool = ctx.enter_context(tc.tile_pool(name="p_ur", bufs=2, space="PSUM"))
    p_g_pool = ctx.enter_context(tc.tile_pool(name="p_g", bufs=2, space="PSUM"))

    W = {}
    for name in ("Wu", "Wr", "Wh", "Uu", "Ur", "Uh"):
        t = singles.tile([D, D], fmm, tag=name)
        nc.sync.dma_start(t[:], aps[name])
        W[name] = t
    Bv = {}
    for name in ("bu", "br", "bh", "nbu"):
        t = singles.tile([D, 1], f32, tag=name)
        nc.sync.dma_start(t[:], aps[name])
        Bv[name] = t

    h = hpool.tile([D, BS], fmm, tag="h")
    nc.sync.dma_start(h[:], aps["h0T"])

    xT = aps["xT"]
    outT = aps["outT"]

    # blend parts of the previous step (h' = q1 + q2); feeding them to the
    # r-projection separately lets Ur@q2 start before the h' add finishes,
    # pulling the chain-head matmul partially off the critical loop.
    q1_prev = q2_prev = None

    for t_step in range(n_steps):
        xt = xpool.tile([D, BS], fmm, tag="x")
        nc.sync.dma_start(xt[:], xT[t_step])

        p_u = p_ur_pool.tile([D, BS], f32, tag="pu")
        p_r = p_ur_pool.tile([D, BS], f32, tag="pr")
        p_zh = p_g_pool.tile([D, BS], f32, tag="zh")
        p_hh = p_g_pool.tile([D, BS], f32, tag="hh")

        # x projections (independent of h; start each PSUM group, so they
        # must execute before the matching h-matmul accumulate)
        nc.tensor.matmul(p_r[:], W["Wr"][:], xt[:], start=True, stop=False)
        nc.tensor.matmul(p_zh[:], W["Wh"][:], xt[:], start=True, stop=True)
        nc.tensor.matmul(p_u[:], W["Wu"][:], xt[:], start=True, stop=False)
        # r-projection first: it heads the serial chain (sigma_r -> m -> z
        # -> tanh). Split over (q1, q2) when available.
        if q1_prev is None:
            nc.tensor.matmul(p_r[:], W["Ur"][:], h[:], start=False, stop=True)
        else:
            nc.tensor.matmul(p_r[:], W["Ur"][:], q1_prev[:], start=False, stop=False)
            nc.tensor.matmul(p_r[:], W["Ur"][:], q2_prev[:], start=False, stop=True)
        nc.tensor.matmul(p_hh[:], W["Uh"][:], h[:], start=True, stop=True)
        nc.tensor.matmul(p_u[:], W["Uu"][:], h[:], start=False, stop=True)

        r = tmp.tile([D, BS], f32, tag="r")
        nc.scalar.activation(r[:], p_r[:], Sigmoid, bias=Bv["br"][:])
        u = tmp.tile([D, BS], f32, tag="u")
        nc.scalar.activation(u[:], p_u[:], Sigmoid, bias=Bv["bu"][:])

        m = tmp.tile([D, BS], f32, tag="m")
        nc.vector.tensor_mul(m[:], r[:], p_hh[:])
        z = tmp.tile([D, BS], f32, tag="z")
        nc.vector.tensor_add(z[:], m[:], p_zh[:])

        hh = tmp.tile([D, BS], f32, tag="hh")
        nc.scalar.activation(hh[:], z[:], Tanh, bias=Bv["bh"][:])

        # blend: h' = (1-u)*h + u*hh; um/q1 run off the tanh chain.
        # um = 1 - u on the DVE (tensor_scalar, 2x mode) keeps ACT free so
        # tanh isn't delayed behind a third sigmoid.
        um = tmp.tile([D, BS], f32, tag="um")
        nc.vector.tensor_scalar(
            um[:], u[:], -1.0, 1.0, mybir.AluOpType.mult, mybir.AluOpType.add
        )
        q1 = tmp.tile([D, BS], fmm, tag="q1")
        nc.vector.tensor_mul(q1[:], um[:], h[:])
        q2 = tmp.tile([D, BS], fmm, tag="q2")
        nc.vector.tensor_mul(q2[:], u[:], hh[:])
        h_new = hpool.tile([D, BS], fmm, tag="h")
        nc.vector.tensor_add(h_new[:], q1[:], q2[:])

        nc.sync.dma_start(outT[t_step], h_new[:])
        h = h_new
        q1_prev, q2_prev = q1, q2


def build_module(n_steps=S):
    import concourse.bacc as bacc
    import concourse.tile as tile
    from concourse import mybir

    f32 = mybir.dt.float32
    fmm = mybir.dt.float32r if USE_F32R else f32
    nc = bacc.Bacc(
        "TRN2",
        target_bir_lowering=False,
        debug=False,
        enable_asserts=False,
        num_devices=NCORES,
    )

    aps = {}
    aps["xT"] = nc.dram_tensor("xT", [n_steps, D, BS], fmm, kind="ExternalInput").ap()
    aps["h0T"] = nc.dram_tensor("h0T", [D, BS], fmm, kind="ExternalInput").ap()
    for name in ("Wu", "Wr", "Wh", "Uu", "Ur", "Uh"):
        aps[name] = nc.dram_tensor(name, [D, D], fmm, kind="ExternalInput").ap()
    for name in ("bu", "br", "bh", "nbu"):
        aps[name] = nc.dram_tensor(name, [D, 1], f32, kind="ExternalInput").ap()
    aps["outT"] = nc.dram_tensor(
        "outT", [n_steps, D, BS], fmm, kind="ExternalOutput"
    ).ap()

    with tile.TileContext(nc) as tc, ExitStack() as ctx:
        _body(ctx, tc, aps, n_steps)
    nc.compile()
    return nc


def _get_built():
    global _BUILT
    if _BUILT is None:
        _BUILT = build_module(S)
    return _BUILT


def _shard_core(c, x, h0):
    xc = x[c * BS : (c + 1) * BS]  # [BS, S, D]
    xT = np.empty((S, D, BS), dtype=np.float32)
    for t in range(S):
        xT[t] = xc[:, t, :].T
    h0T = np.ascontiguousarray(h0[c * BS : (c + 1) * BS].T)
    return xT, h0T


def _prep_in_maps(inputs):
    x = np.ascontiguousarray(np.asarray(inputs["x"], dtype=np.float32))
    h0 = np.ascontiguousarray(np.asarray(inputs["h0"], dtype=np.float32))
    shared = {}
    for name in ("Wu", "Wr", "Wh", "Uu", "Ur", "Uh"):
        shared[name] = np.ascontiguousarray(np.asarray(inputs[name], dtype=np.float32))
    for name in ("bu", "br", "bh"):
        shared[name] = np.ascontiguousarray(
            np.asarray(inputs[name], dtype=np.float32).reshape(1, D).T
        )
    shared["nbu"] = np.ascontiguousarray(-shared["bu"])
    with ThreadPoolExecutor(NCORES) as ex:
        parts = list(ex.map(lambda c: _shard_core(c, x, h0), range(NCORES)))

    in_maps = []
    for c in range(NCORES):
        xT, h0T = parts[c]
        m = {"xT": xT, "h0T": h0T}
        m.update(shared)
        in_maps.append(m)
    return in_maps


def _unshard_core(c, outT, outs):
    # outT: [S, D, BS] -> outs[c*BS:(c+1)*BS] = [BS, S, D]
    dst = outs[c * BS : (c + 1) * BS]
    for t in range(S):
        dst[:, t, :] = outT[t].T


def _assemble(results):
    outs = np.empty((B, S, D), dtype=np.float32)
    with ThreadPoolExecutor(NCORES) as ex:
        list(
            ex.map(
                lambda c: _unshard_core(c, results[c]["outT"], outs), range(NCORES)
            )
        )
    h_last = np.ascontiguousarray(outs[:, -1, :])
    return outs, h_last


def _ensure_ntff_hook():
    """Install the axon NTFF profile hook if the image's antenv lacks it."""
    try:
        from antenv.axon_hooks import get_axon_ntff_profile_hook  # noqa: F401

        return True
    except ImportError:
        pass
    try:
        import types

        import antenv
        from trn_agent_boot.trn_boot import _ntff_profile_via_ctypes

        hook = _ntff_profile_via_ctypes("/opt/axon/libaxon_pjrt.so")
        mod = types.ModuleType("antenv.axon_hooks")
        mod.get_axon_ntff_profile_hook = lambda: hook
        mod.set_axon_ntff_profile_hook = lambda h: None
        sys.modules["antenv.axon_hooks"] = mod
        antenv.axon_hooks = mod
        return hook is not None
    except Exception as e:  # pragma: no cover
        print(f"NTFF hook install failed: {e}", file=sys.stderr)
        return False


def run(inputs, trace=False):
    """Run on hardware; returns ((outs, h_last), exec_time_ns_or_None)."""
    import concourse.bass_utils as bass_utils

    if trace:
        _ensure_ntff_hook()
        bass_utils.upload_artifacts = lambda tmpdir: f"local:{tmpdir}"

    nc = _get_built()
    in_maps = _prep_in_maps(inputs)
    res = bass_utils.run_bass_kernel_spmd(
        nc, in_maps, core_ids=list(range(NCORES)), trace=trace
    )
    return _assemble(res.results), res.exec_time_ns


def kernel(**inputs):
    (outs, h_last), _ = run(inputs, trace=False)
    return outs, h_last


# revision 26
# speedup vs baseline: 1.2888x; 1.0032x over previous
"""DIEN GRU-with-attention kernel for Trainium2 (8 NeuronCores, Bass/Tile).

Math note: the reference computes softmax over a singleton axis, which is
exactly 1.0, so the attention branch (Wa, item) never affects the output.
The computation reduces to a plain GRU:
    u  = sigmoid(x_t @ Wu + h @ Uu + bu)
    r  = sigmoid(x_t @ Wr + h @ Ur + br)
    hh = tanh(x_t @ Wh + r * (h @ Uh) + bh)
    h' = (1 - u) * h + u * hh

Device layout is feature-major: tiles are [D=128 partitions, batch free].
The batch dim (2048) is sharded 8 ways (256 per core); the host does the
[batch, feat] <-> [feat, batch] layout transposes during shard/unshard.
"""

import sys

if "/opt/trn_rl_repo" not in sys.path:
    sys.path.insert(0, "/opt/trn_rl_repo")

from concurrent.futures import ThreadPoolExecutor
from contextlib import ExitStack

import os

import numpy as np

B, S, D = 2048, 200, 128
NCORES = 8
BS = B // NCORES  # batch per core

# float32r streams the PE at 1 cycle/row (vs 4 for fp32's two-pass lowering)
# but measured 5.6e-2 rel err end-to-end — too lossy. Default off.
USE_F32R = os.environ.get("DIEN_F32R", "0") == "1"

_BUILT = None  # cached compiled module


def _body(ctx, tc, aps, n_steps):
    import concourse.bass as bass  # noqa: F401
    from concourse import mybir

    nc = tc.nc
    f32 = mybir.dt.float32
    Sigmoid = mybir.ActivationFunctionType.Sigmoid
    Tanh = mybir.ActivationFunctionType.Tanh

    fmm = mybir.dt.float32r if USE_F32R else f32

    singles = ctx.enter_context(tc.tile_pool(name="singles", bufs=1))
    xpool = ctx.enter_context(tc.tile_pool(name="xp", bufs=4))
    hpool = ctx.enter_context(tc.tile_pool(name="hp", bufs=3))
    tmp = ctx.enter_context(tc.tile_pool(name="tmp", bufs=3))
    p_ur_pool = ctx.enter_context(tc.tile_pool(name="p_ur", bufs=2, space="PSUM"))
    p_g_pool = ctx.enter_context(tc.tile_pool(name="p_g", bufs=2, space="PSUM"))

    W = {}
    for name in ("Wu", "Wr", "Wh", "Uu", "Ur", "Uh"):
        t = singles.tile([D, D], fmm, tag=name)
        nc.sync.dma_start(t[:], aps[name])
        W[name] = t
    Bv = {}
    for name in ("bu", "br", "bh", "nbu"):
        t = singles.tile([D, 1], f32, tag=name)
        nc.sync.dma_start(t[:], aps[name])
        Bv[name] = t

    h = hpool.tile([D, BS], fmm, tag="h")
    nc.sync.dma_start(h[:], aps["h0T"])

    xT = aps["xT"]
    outT = aps["outT"]

    # Software-pipelined emission: the matmuls for step t+1 are emitted
    # immediately after their actual producers in step t (x-projections and
    # Ur@q1 after the q1 op, Ur@q2 after q2, Uh/Uu after h'), so each gets
    # its own semaphore wait instead of one coalesced wait on the latest
    # producer. h' = q1 + q2, and Ur@(q1)+Ur@(q2) accumulate separately so
    # the r-projection starts before the h' add completes.
    def emit_x_and_rq1(t_next, q1_tile):
        xt = xpool.tile([D, BS], fmm, tag="x")
        nc.sync.dma_start(xt[:], xT[t_next])
        t_pu = p_ur_pool.tile([D, BS], f32, tag="pu")
        t_pr = p_ur_pool.tile([D, BS], f32, tag="pr")
        t_pzh = p_g_pool.tile([D, BS], f32, tag="zh")
        t_phh = p_g_pool.tile([D, BS], f32, tag="hh")
        p = {"u": t_pu, "r": t_pr, "zh": t_pzh, "hh": t_phh}
        nc.tensor.matmul(p["r"][:], W["Wr"][:], xt[:], start=True, stop=False)
        nc.tensor.matmul(p["zh"][:], W["Wh"][:], xt[:], start=True, stop=True)
        nc.tensor.matmul(p["u"][:], W["Wu"][:], xt[:], start=True, stop=False)
        if q1_tile is not None:
            nc.tensor.matmul(p["r"][:], W["Ur"][:], q1_tile[:], start=False, stop=False)
        return p

    # prologue: step 0 projections from h0 directly
    p_cur = emit_x_and_rq1(0, None)
    nc.tensor.matmul(p_cur["r"][:], W["Ur"][:], h[:], start=False, stop=True)
    nc.tensor.matmul(p_cur["hh"][:], W["Uh"][:], h[:], start=True, stop=True)
    nc.tensor.matmul(p_cur["u"][:], W["Uu"][:], h[:], start=False, stop=True)

    for t_step in range(n_steps):
        has_next = t_step + 1 < n_steps

        r = tmp.tile([D, BS], f32, tag="r")
        nc.scalar.activation(r[:], p_cur["r"][:], Sigmoid, bias=Bv["br"][:])
        u = tmp.tile([D, BS], f32, tag="u")
        nc.scalar.activation(u[:], p_cur["u"][:], Sigmoid, bias=Bv["bu"][:])

        m = tmp.tile([D, BS], f32, tag="m")
        nc.vector.tensor_mul(m[:], r[:], p_cur["hh"][:])
        z = tmp.tile([D, BS], f32, tag="z")
        nc.vector.tensor_add(z[:], m[:], p_cur["zh"][:])

        # um = 1 - u on the DVE (tensor_scalar, 2x mode) keeps ACT free so
        # tanh isn't delayed behind a third sigmoid.
        um = tmp.tile([D, BS], f32, tag="um")
        nc.vector.tensor_scalar(
            um[:], u[:], -1.0, 1.0, mybir.AluOpType.mult, mybir.AluOpType.add
        )
        q1 = tmp.tile([D, BS], fmm, tag="q1")
        nc.vector.tensor_mul(q1[:], um[:], h[:])
        if has_next:
            p_nxt = emit_x_and_rq1(t_step + 1, q1)

        hh = tmp.tile([D, BS], f32, tag="hh")
        nc.scalar.activation(hh[:], z[:], Tanh, bias=Bv["bh"][:])

        q2 = tmp.tile([D, BS], fmm, tag="q2")
        nc.vector.tensor_mul(q2[:], u[:], hh[:])
        if has_next:
            nc.tensor.matmul(p_nxt["r"][:], W["Ur"][:], q2[:], start=False, stop=True)

        h_new = hpool.tile([D, BS], fmm, tag="h")
        nc.vector.tensor_add(h_new[:], q1[:], q2[:])
        nc.sync.dma_start(outT[t_step], h_new[:])
        if has_next:
            nc.tensor.matmul(p_nxt["hh"][:], W["Uh"][:], h_new[:], start=True, stop=True)
            nc.tensor.matmul(p_nxt["u"][:], W["Uu"][:], h_new[:], start=False, stop=True)
            p_cur = p_nxt
        h = h_new


def build_module(n_steps=S):
    import concourse.bacc as bacc
    import concourse.tile as tile
    from concourse import mybir

    f32 = mybir.dt.float32
    fmm = mybir.dt.float32r if USE_F32R else f32
    nc = bacc.Bacc(
        "TRN2",
        target_bir_lowering=False,
        debug=False,
        enable_asserts=False,
        num_devices=NCORES,
    )

    aps = {}
    aps["xT"] = nc.dram_tensor("xT", [n_steps, D, BS], fmm, kind="ExternalInput").ap()
    aps["h0T"] = nc.dram_tensor("h0T", [D, BS], fmm, kind="ExternalInput").ap()
    for name in ("Wu", "Wr", "Wh", "Uu", "Ur", "Uh"):
        aps[name] = nc.dram_tensor(name, [D, D], fmm, kind="ExternalInput").ap()
    for name in ("bu", "br", "bh", "nbu"):
        aps[name] = nc.dram_tensor(name, [D, 1], f32, kind="ExternalInput").ap()
    aps["outT"] = nc.dram_tensor(
        "outT", [n_steps, D, BS], fmm, kind="ExternalOutput"
    ).ap()

    with tile.TileContext(nc) as tc, ExitStack() as ctx:
        _body(ctx, tc, aps, n_steps)
    nc.compile()
    return nc


def _get_built():
    global _BUILT
    if _BUILT is None:
        _BUILT = build_module(S)
    return _BUILT


def _shard_core(c, x, h0):
    xc = x[c * BS : (c + 1) * BS]  # [BS, S, D]
    xT = np.empty((S, D, BS), dtype=np.float32)
    for t in range(S):
        xT[t] = xc[:, t, :].T
    h0T = np.ascontiguousarray(h0[c * BS : (c + 1) * BS].T)
    return xT, h0T


def _prep_in_maps(inputs):
    x = np.ascontiguousarray(np.asarray(inputs["x"], dtype=np.float32))
    h0 = np.ascontiguousarray(np.asarray(inputs["h0"], dtype=np.float32))
    shared = {}
    for name in ("Wu", "Wr", "Wh", "Uu", "Ur", "Uh"):
        shared[name] = np.ascontiguousarray(np.asarray(inputs[name], dtype=np.float32))
    for name in ("bu", "br", "bh"):
        shared[name] = np.ascontiguousarray(
            np.asarray(inputs[name], dtype=np.float32).reshape(1, D).T
        )
    shared["nbu"] = np.ascontiguousarray(-shared["bu"])
    with ThreadPoolExecutor(NCORES) as ex:
        parts = list(ex.map(lambda c: _shard_core(c, x, h0), range(NCORES)))

    in_maps = []
    for c in range(NCORES):
        xT, h0T = parts[c]
        m = {"xT": xT, "h0T": h0T}
        m.update(shared)
        in_maps.append(m)
    return in_maps


def _unshard_core(c, outT, outs):
    # outT: [S, D, BS] -> outs[c*BS:(c+1)*BS] = [BS, S, D]
    dst = outs[c * BS : (c + 1) * BS]
    for t in range(S):
        dst[:, t, :] = outT[t].T


def _assemble(results):
    outs = np.empty((B, S, D), dtype=np.float32)
    with ThreadPoolExecutor(NCORES) as ex:
        list(
            ex.map(
                lambda c: _unshard_core(c, results[c]["outT"], outs), range(NCORES)
            )
        )
    h_last = np.ascontiguousarray(outs[:, -1, :])
    return outs, h_last


def _ensure_ntff_hook():
    """Install the axon NTFF profile hook if the image's antenv lacks it."""
    try:
        from antenv.axon_hooks import get_axon_ntff_profile_hook  # noqa: F401

        return True
    except ImportError:
        pass
    try:
        import types

        import antenv
        from trn_agent_boot.trn_boot import _ntff_profile_via_ctypes

        hook = _ntff_profile_via_ctypes("/opt/axon/libaxon_pjrt.so")
        mod = types.ModuleType("antenv.axon_hooks")
        mod.get_axon_ntff_profile_hook = lambda: hook
        mod.set_axon_ntff_profile_hook = lambda h: None
        sys.modules["antenv.axon_hooks"] = mod
        antenv.axon_hooks = mod
        return hook is not None
    except Exception as e:  # pragma: no cover
        print(f"NTFF hook install failed: {e}", file=sys.stderr)
        return False


def run(inputs, trace=False):
    """Run on hardware; returns ((outs, h_last), exec_time_ns_or_None)."""
    import concourse.bass_utils as bass_utils

    if trace:
        _ensure_ntff_hook()
        bass_utils.upload_artifacts = lambda tmpdir: f"local:{tmpdir}"

    nc = _get_built()
    in_maps = _prep_in_maps(inputs)
    res = bass_utils.run_bass_kernel_spmd(
        nc, in_maps, core_ids=list(range(NCORES)), trace=trace
    )
    return _assemble(res.results), res.exec_time_ns


def kernel(**inputs):
    (outs, h_last), _ = run(inputs, trace=False)
    return outs, h_last
